# revision 5
# baseline (speedup 1.0000x reference)
"""AdaModConv1D on 8 TRN2 NeuronCores — int8/hybrid transport, v4.

Per-core layout: the 32768-col half-stacked sequence is split into 9 chunks:
two leading 2048-col bf16 chunks (arrive fast, need no dequant, so the PE
starts ~4us in), five 4096-col int8 chunks (DVE dequantizes at 2x cast rate
while the DMA streams), and two trailing 4096-col bf16 chunks (no dequant in
the pipeline tail).  Output is int8 (scale folded into host-computed
modulated weights; PSUM->SBUF copies cast with round-to-nearest).  PSUM runs
4 rotating [128,1024] tiles so the copy latency (ACT ~1.0us, DVE ~1.2us per
tile) never serializes with the PE (one window-pair = 12 matmuls per tile).

Resource balance per core: DMA ~27us (10MB at ~23.4GB/s/engine x16),
ACT ~23us (23 copies), DVE ~23us (10 dequant atoms + 9 copies), PE ~24us.
"""

import os
import sys

sys.path.insert(0, "/opt/trn_rl_repo")

import numpy as np
import ml_dtypes

BF16 = ml_dtypes.bfloat16

B, L, C = 8, 65536, 64
F, KW, DL = 64, 3, 256
EPS = 1e-8
H = L // 2            # 32768 per partition-half
OUT_SCALE = 22.0
NPIECE = 16           # output DMA pieces of 2048 cols

# chunk table: (width, kind); bases are cumulative.  kind: 0=bf16, 1=int8
CHUNKS = [(2048, 0), (2048, 0)] + [(4096, 1)] * 5 + [(4096, 0)] * 2
NCH = len(CHUNKS)
I8_CHUNKS = [i for i, (_, k) in enumerate(CHUNKS) if k == 1]
BF_CHUNKS = [i for i, (_, k) in enumerate(CHUNKS) if k == 0]
NTILE = H // 1024     # 32 PSUM tiles / copies

# PSUM->SBUF copies on DVE (rest on ACT); tile NTILE-1 is split half/half
DVE_COPIES = {13, 15, 17, 19, 21, 23, 25, 27, 29}

_cached = {}


def _chunk_base(i):
    return sum(w for w, _ in CHUNKS[:i])


def _build():
    import concourse.bass as bass
    import concourse.bacc as bacc
    import concourse.mybir as mybir
    import concourse.tile as tile

    dt = mybir.dt
    nc = bacc.Bacc("TRN2", target_bir_lowering=False, debug=False, num_devices=8)

    # DRAM inputs: one param per chunk (host lays out contiguous [128, W+2])
    xdram = []
    for i, (w, kind) in enumerate(CHUNKS):
        xdram.append(
            nc.declare_dram_parameter(
                f"x{i}", [128, w + 2], dt.int8 if kind else dt.bfloat16,
                isOutput=False,
            )
        )
    par = nc.declare_dram_parameter("par", [128, KW * F], dt.bfloat16, isOutput=False)
    yout = nc.declare_dram_parameter(
        "yout", [NPIECE, 128, 2048], dt.int8, isOutput=True
    )

    with tile.TileContext(nc) as tc:
        with (
            tc.tile_pool(name="xin", bufs=1) as xin_pool,
            tc.tile_pool(name="xb", bufs=3) as xb_pool,
            tc.tile_pool(name="yout", bufs=1) as yout_pool,
            tc.tile_pool(name="pre", bufs=1) as pre,
            tc.tile_pool(name="cp", bufs=4, space="PSUM") as conv_psum,
        ):
            wfin = pre.tile([128, KW * F], dt.bfloat16, tag="wfin")
            nc.sync.dma_start(out=wfin[:], in_=par[:])

            # PE warm-up: the clock needs ~3us of continuous execution to
            # reach full rate; run dummy matmuls on garbage data until the
            # first real chunk lands so the conv starts on a hot array
            g = pre.tile([128, 576], dt.bfloat16, tag="warm")
            nc.vector.memset(g[:], 0.0)
            Pw = conv_psum.tile([128, 512], dt.float32, name="pswarm", tag="convps")
            for _ in range(10):
                nc.tensor.matmul(
                    Pw[0:64, :], lhsT=g[:, 0:64], rhs=g[:, 64:576],
                    start=True, stop=True, skip_group_check=True,
                )

            xq = [None] * NCH   # raw int8 tiles
            xb = [None] * NCH   # bf16 tiles the PE reads
            xtile = []
            for i, (w, kind) in enumerate(CHUNKS):
                t = xin_pool.tile(
                    [128, w + 2], dt.int8 if kind else dt.bfloat16, tag=f"xi{i}"
                )
                xtile.append(t)
                if kind:
                    xq[i] = t
                else:
                    xb[i] = t

            # issue order: chunk 2 (first int8) lands before chunk 1 so its
            # dequant starts ~1.4us earlier — the PE consumes the bf16 leads
            # hot and otherwise stalls at the lead->int8 transition
            for i in [0, 2, 1] + list(range(3, NCH)):
                nc.sync.dma_start(out=xtile[i][:], in_=xdram[i][:])

            yc = yout_pool.tile([128, H], dt.int8, tag="yc")

            def dequant(i):
                w = CHUNKS[i][0]
                t = xb_pool.tile([128, w + 2], dt.bfloat16, tag="xb")
                half = w // 2 + 2
                # two atoms so DVE can interleave copies between them
                nc.vector.tensor_copy(t[:, 0:half], xq[i][:, 0:half])
                nc.vector.tensor_copy(t[:, half:], xq[i][:, half:])
                xb[i] = t

            first_i8 = I8_CHUNKS[0]
            tile_idx = 0
            for i, (w, kind) in enumerate(CHUNKS):
                if i == 0 and kind:
                    dequant(i)
                if i + 1 < NCH and CHUNKS[i + 1][1]:
                    dequant(i + 1)
                x = xb[i]
                base = _chunk_base(i)
                for t in range(w // 1024):
                    g = base + t * 1024          # global col of this tile
                    u = t * 1024                  # local col within chunk
                    P = conv_psum.tile(
                        [128, 1024], dt.float32, name=f"ps{tile_idx}", tag="convps"
                    )
                    b0, b1 = u, u + 512
                    for k in range(KW):
                        st, sp = (k == 0), (k == KW - 1)
                        wA = wfin[0:64, k * F : (k + 1) * F]
                        wB = wfin[64:128, k * F : (k + 1) * F]
                        nc.tensor.matmul(
                            P[0:64, 0:512],
                            lhsT=wA, rhs=x[0:64, b0 + k : b0 + k + 512],
                            start=st, stop=sp, skip_group_check=True,
                        )
                        nc.tensor.matmul(
                            P[64:128, 0:512],
                            lhsT=wB, rhs=x[64:128, b0 + k : b0 + k + 512],
                            start=st, stop=sp, skip_group_check=True,
                        )
                        nc.tensor.matmul(
                            P[64:128, 512:1024],
                            lhsT=wA, rhs=x[0:64, b1 + k : b1 + k + 512],
                            start=st, stop=sp, skip_group_check=True,
                        )
                        nc.tensor.matmul(
                            P[0:64, 512:1024],
                            lhsT=wB, rhs=x[64:128, b1 + k : b1 + k + 512],
                            start=st, stop=sp, skip_group_check=True,
                        )
                    dst = yc[:, g : g + 1024]
                    if tile_idx == NTILE - 1:
                        nc.scalar.copy(dst[:, 0:512], P[:, 0:512])
                        nc.vector.tensor_copy(dst[:, 512:1024], P[:, 512:1024])
                    elif tile_idx in DVE_COPIES:
                        nc.vector.tensor_copy(dst, P[:])
                    else:
                        nc.scalar.copy(dst, P[:])
                    if tile_idx % 2 == 1:
                        piece = tile_idx // 2
                        pg = piece * 2048
                        if piece == NPIECE - 1:
                            for q in (0, 1024):
                                nc.sync.dma_start(
                                    out=yout[piece, :, q : q + 1024],
                                    in_=yc[:, pg + q : pg + q + 1024],
                                )
                        else:
                            nc.sync.dma_start(
                                out=yout[piece], in_=yc[:, pg : pg + 2048]
                            )
                    tile_idx += 1

    nc.compile()
    return nc


def _get_nc():
    if "nc" not in _cached:
        _cached["nc"] = _build()
    return _cached["nc"]


def _softplus(v):
    return np.logaddexp(0.0, v)


def make_wfin(ltnt_b, kernel, Wd, bd, qs):
    """Host-side modulated weights [128, KW*F] bf16 incl transport scales."""
    p = ltnt_b @ Wd + bd
    s = _softplus(p) + 1.0
    d = 1.0 / np.sqrt(np.einsum("kcf,c->f", kernel * kernel, s * s) + EPS)
    w = kernel * s[None, :, None] * d[None, None, :]
    w = w * np.float32(OUT_SCALE / qs)
    blk = w.transpose(1, 0, 2).reshape(64, KW * F)
    return np.tile(blk, (2, 1)).astype(BF16)


def make_xin(data_b, qs):
    """Chunked layout of one sample scaled by qs (int8 rounded, bf16 raw)."""
    xs = data_b * qs
    xt = xs.reshape(2, H, C).transpose(0, 2, 1)
    flat = np.zeros((128, H + 2), dtype=np.float32)
    flat[:, 1 : H + 1] = xt.reshape(128, H)
    flat[64:128, 0] = xt[0, :, -1]
    flat[0:64, H + 1] = xt[1, :, 0]
    chunks = {}
    for i, (w, kind) in enumerate(CHUNKS):
        b = _chunk_base(i)
        sl = flat[:, b : b + w + 2]
        if kind:
            chunks[f"x{i}"] = np.clip(np.rint(sl), -127, 127).astype(np.int8)
        else:
            chunks[f"x{i}"] = sl.astype(BF16)
    return chunks


def build_inmaps(data, ltnt, kernel_w, Wd, bd):
    in_maps, descales = [], []
    for b in range(B):
        amax = float(np.abs(data[b]).max())
        qs = 127.0 / amax if amax > 0 else 1.0
        m = make_xin(data[b], qs)
        m["par"] = make_wfin(ltnt[b], kernel_w, Wd, bd, qs)
        in_maps.append(m)
        descales.append(1.0 / OUT_SCALE)
    return in_maps, descales


def kernel(data, ltnt, kernel, Wd, bd):
    if "jax" not in sys.modules:
        plats = os.environ.get("JAX_PLATFORMS", "")
        if plats and "axon" not in plats:
            os.environ["JAX_PLATFORMS"] = "axon," + plats

    from concourse import bass_utils

    nc = _get_nc()

    data = np.asarray(data, dtype=np.float32)
    ltnt = np.asarray(ltnt, dtype=np.float32)
    kf = np.asarray(kernel, dtype=np.float32)
    wdf = np.asarray(Wd, dtype=np.float32)
    bdf = np.asarray(bd, dtype=np.float32)

    in_maps, descales = build_inmaps(data, ltnt, kf, wdf, bdf)

    try:
        res = bass_utils.run_bass_kernel_spmd(nc, in_maps, core_ids=list(range(B)))
    except Exception:
        import time

        time.sleep(15)
        res = bass_utils.run_bass_kernel_spmd(nc, in_maps, core_ids=list(range(B)))

    out = np.empty((B, L, C), dtype=np.float32)
    NGRP = H // 512
    even = (np.arange(NGRP) % 2 == 0)[None, :, None]
    for b in range(B):
        yp = np.asarray(res.results[b]["yout"]).astype(np.float32) * descales[b]
        yo = yp.transpose(1, 0, 2).reshape(128, H)
        yr = yo.reshape(2, F, NGRP, 512)
        h0 = np.where(even, yr[0], yr[1])
        h1 = np.where(even, yr[1], yr[0])
        out[b, :H] = h0.transpose(1, 2, 0).reshape(H, F)
        out[b, H:] = h1.transpose(1, 2, 0).reshape(H, F)
    return out
